# revision 1
# baseline (speedup 1.0000x reference)
"""Multi-head attention block (B=4, N=2048, D=1024, H=16) on 8 trn2 NeuronCores.

Sharding: core c -> (batch b = c//2, head-group g = c%2) with 8 heads per
group.  Each core computes q/k/v for its 8 heads over its batch, full
attention, and a partial projection y_part = attn_out_g @ w_proj[rows_g].
Host combines: out[b] = y_part[2b] + y_part[2b+1] + b_proj.

Dataflow on device is "transposed": q', k' live as [head_dim, seq] so the
PE array (out = lhsT.T @ rhs, contraction over partitions) never needs
transposed activations except for one PE-transpose of x at the start.

All matmuls run in float32r (fp32 data, full PE rate). fp32r operands must
be written by a compute engine as float32r (rounding); operands at base
partition 64 crash the fp32r path on hardware, so every fp32r matmul
operand here sits at partitions 0..63 or spans the full 128 partitions.
q'/k' bounce through DRAM scratch to get a head-major base-0 layout.
"""
import sys

sys.path.insert(0, "/opt/trn_rl_repo")

import numpy as np

import concourse.bass as bass
import concourse.mybir as mybir
import concourse.tile as tile
from concourse import bacc
from concourse.bass_utils import run_bass_kernel_spmd
from concourse.masks import make_identity

F32 = mybir.dt.float32
F32R = mybir.dt.float32r
AF = mybir.ActivationFunctionType

B = 4            # batch
N = 2048         # sequence length
D = 1024         # model dim
H = 16           # total heads
HD = 64          # head dim
HL = 8           # heads per core (local)
SCALE = HD ** -0.5

NKT = N // 128   # 16 key tiles
NRW = N // 512   # 4 row windows
NDT = D // 128   # 8 d tiles
QWIN = 1024      # q window in attention phase
NQW = N // QWIN  # 2


def _build_nc(rep=1):
    nc = bacc.Bacc(None, target_bir_lowering=False)

    x = nc.declare_dram_parameter("x", [N, D], F32, isOutput=False)
    wqk = nc.declare_dram_parameter("wqk", [D, D], F32, isOutput=False)
    wv = nc.declare_dram_parameter("wv", [D, 512], F32, isOutput=False)
    wp = nc.declare_dram_parameter("wp", [512, D], F32, isOutput=False)
    y = nc.declare_dram_parameter("y", [N, D], F32, isOutput=True)

    with tile.TileContext(nc) as tc:
      with tc.tile_pool(name="dramp", bufs=1, space="DRAM") as dramp:
        # DRAM scratch: q'/k' head-major [h][hd, seq] (f32r-rounded on write),
        # denominators for the broadcast bounce. Pool tiles (not raw
        # dram_tensor) so Tile tracks write->read dependencies.
        qc = [dramp.tile([128, N], F32R, tag=f"qc{p}", name=f"qc{p}")
              for p in range(HL // 2)]
        kc = [dramp.tile([128, N], F32R, tag=f"kc{p}", name=f"kc{p}")
              for p in range(HL // 2)]
        dscr = [dramp.tile([NQW, QWIN], F32, tag=f"dscr{h}", name=f"dscr{h}")
                for h in range(HL)]
        for _rep in range(rep):
         with tc.tile_pool(name="consts", bufs=1) as consts, \
             tc.tile_pool(name="resv", bufs=1) as resv:
            ident = consts.tile([128, 128], F32, tag="ident")
            make_identity(nc, ident)
            ones8 = consts.tile([128, HL], F32, tag="ones8")
            nc.vector.memset(ones8, 1.0)

            # v_aug[kt]: [128 keys, HL*(HD+1)]; per head: 64 v cols + ones col
            v_aug = [resv.tile([128, HL * (HD + 1)], F32R, tag=f"va{kt}", name=f"va{kt}")
                     for kt in range(NKT)]

            # ---------------- Phase A: QKV ----------------
            # Order matters for overlap with phase B: transpose x, then v
            # (unlocks PV), then q'/k' head-major (unlocks attention per
            # head-pair while the rest of QKV still runs).
            with tc.tile_pool(name="wqk_sb", bufs=2) as wqk_sb, \
                 tc.tile_pool(name="wres", bufs=1) as wres, \
                 tc.tile_pool(name="xst", bufs=2) as xst, \
                 tc.tile_pool(name="xr", bufs=1) as xrp, \
                 tc.tile_pool(name="qkst", bufs=2) as qkst, \
                 tc.tile_pool(name="tp_ps", bufs=4, space="PSUM") as tp_ps, \
                 tc.tile_pool(name="qk_ps", bufs=2, space="PSUM") as qk_ps:

                # load + round weights
                wqk_r, wv_r = [], []
                for dt in range(NDT):
                    st = wqk_sb.tile([128, D], F32, tag="wqkst")
                    nc.sync.dma_start(out=st, in_=wqk[dt * 128:(dt + 1) * 128, :])
                    wr = wres.tile([128, D], F32R, tag=f"wqk{dt}")
                    nc.vector.tensor_copy(wr, st)
                    wqk_r.append(wr)
                    st2 = wqk_sb.tile([128, 512], F32, tag="wvst")
                    nc.sync.dma_start(out=st2, in_=wv[dt * 128:(dt + 1) * 128, :])
                    wr2 = wres.tile([128, 512], F32R, tag=f"wv{dt}")
                    nc.vector.tensor_copy(wr2, st2)
                    wv_r.append(wr2)

                # transpose all of x into resident X' [dt][128, N]
                xp = [xrp.tile([128, N], F32R, tag=f"xp{dt}", name=f"xp{dt}")
                      for dt in range(NDT)]
                for rt in range(N // 128):
                    strip = xst.tile([128, D], F32, tag="xstrip")
                    nc.sync.dma_start(out=strip, in_=x[rt * 128:(rt + 1) * 128, :])
                    for dt in range(NDT):
                        tp = tp_ps.tile([128, 128], F32, tag="tp")
                        nc.tensor.transpose(
                            tp, strip[:, dt * 128:(dt + 1) * 128], ident)
                        nc.vector.tensor_copy(
                            xp[dt][:, rt * 128:(rt + 1) * 128], tp)

                # v for all 16 key tiles (unblocks PV matmuls early)
                for kt in range(NKT):
                    ps = qk_ps.tile([128, 512], F32, tag="vps")
                    for dt in range(NDT):
                        nc.tensor.matmul(
                            ps,
                            xp[dt][:, kt * 128:(kt + 1) * 128],
                            wv_r[dt],
                            start=(dt == 0), stop=(dt == NDT - 1))
                    va3 = v_aug[kt].rearrange("p (h c) -> p h c", h=HL)
                    ps3 = ps.rearrange("p (h c) -> p h c", h=HL)
                    nc.vector.tensor_copy(va3[:, :, 0:HD], ps3)
                    nc.vector.tensor_copy(
                        va3[:, :, HD:HD + 1],
                        ones8.rearrange("p (h c) -> p h c", h=HL))

                # q'/k' head-major: k then q for head-pair 0 first, etc.
                for hp in range(4):
                    for sec in (1, 0):          # k' section first, then q'
                        i = (4 + hp) if sec == 1 else hp
                        st = qkst.tile([128, N], F32R, tag="qkstage")
                        for rw in range(NRW):
                            ps = qk_ps.tile([128, 512], F32, tag="qkps")
                            for dt in range(NDT):
                                nc.tensor.matmul(
                                    ps,
                                    wqk_r[dt][:, i * 128:(i + 1) * 128],
                                    xp[dt][:, rw * 512:(rw + 1) * 512],
                                    start=(dt == 0), stop=(dt == NDT - 1))
                            nc.vector.tensor_copy(
                                st[:, rw * 512:(rw + 1) * 512], ps)
                        dst = kc if sec == 1 else qc
                        nc.sync.dma_start(out=dst[hp][:, :], in_=st)

            # ---------------- Phase B: attention ----------------
            with tc.tile_pool(name="ost", bufs=1) as ostp:
                ostack = [ostp.tile([128, N], F32R, tag=f"os{p}", name=f"os{p}")
                          for p in range(HL // 2)]
                with tc.tile_pool(name="ktp", bufs=2) as ktp, \
                     tc.tile_pool(name="qtp", bufs=3) as qtp, \
                     tc.tile_pool(name="pst", bufs=3) as pstp, \
                     tc.tile_pool(name="epi", bufs=2) as epi, \
                     tc.tile_pool(name="s_ps", bufs=2, space="PSUM") as s_psp, \
                     tc.tile_pool(name="o_ps", bufs=1, space="PSUM") as o_psp:

                    for h in range(HL):
                        half = slice(64 * (h % 2), 64 * (h % 2) + 64)
                        k_t = ktp.tile([64, N], F32R, tag="kt")
                        nc.sync.dma_start(out=k_t, in_=kc[h // 2][half, :])
                        q_th = qtp.tile([64, N], F32R, tag="qt")
                        nc.sync.dma_start(out=q_th, in_=qc[h // 2][half, :])
                        for qw in range(NQW):
                            q_t = q_th[:, qw * QWIN:(qw + 1) * QWIN]
                            o_ps = o_psp.tile([65, QWIN], F32, tag="ops")
                            # flat chunk list: c -> (kt = c//2, qh = c%2);
                            # exp strips of 3 chunks (1536 wide) amortize the
                            # ACT per-instruction overhead; PSUM: 3 banks x2.
                            nch = NKT * 2
                            c = 0
                            while c < nch:
                                w = min(3, nch - c)
                                s_ps = s_psp.tile([128, 3 * 512], F32, tag="sps")
                                for s in range(w):
                                    kt, qh = (c + s) // 2, (c + s) % 2
                                    nc.tensor.matmul(
                                        s_ps[:, s * 512:(s + 1) * 512],
                                        k_t[:, kt * 128:(kt + 1) * 128],
                                        q_t[:, qh * 512:(qh + 1) * 512],
                                        start=True, stop=True)
                                p_sb = pstp.tile([128, 3 * 512], F32R, tag="pstrip")
                                nc.scalar.activation(p_sb[:, 0:w * 512],
                                                     s_ps[:, 0:w * 512],
                                                     AF.Exp, scale=SCALE)
                                for s in range(w):
                                    kt, qh = (c + s) // 2, (c + s) % 2
                                    nc.tensor.matmul(
                                        o_ps[:, qh * 512:(qh + 1) * 512],
                                        v_aug[kt][:, h * (HD + 1):(h + 1) * (HD + 1)],
                                        p_sb[:, s * 512:(s + 1) * 512],
                                        start=(kt == 0), stop=(kt == NKT - 1))
                                c += w
                            # epilogue: snapshot o_ps to SBUF fast (frees the
                            # psum bank), then normalize off the sbuf copy.
                            o_raw = epi.tile([65, QWIN], F32, tag="oraw")
                            nc.vector.tensor_copy(o_raw, o_ps)
                            nc.sync.dma_start(out=dscr[h][qw, :], in_=o_raw[64:65, :])
                            den_b = epi.tile([64, QWIN], F32, tag="denb")
                            bc_ap = bass.AP(
                                tensor=dscr[h].tensor,
                                offset=dscr[h].offset + qw * QWIN,
                                ap=[[0, 64], [1, QWIN]])
                            nc.sync.dma_start(out=den_b, in_=bc_ap)
                            rden = epi.tile([64, QWIN], F32, tag="rden")
                            nc.vector.reciprocal(rden, den_b)
                            qsl = slice(qw * QWIN, (qw + 1) * QWIN)
                            if h % 2 == 0:
                                nc.vector.tensor_mul(
                                    ostack[h // 2][0:64, qsl], o_raw[0:64, :], rden)
                            else:
                                otmp = epi.tile([64, QWIN], F32R, tag="otmp")
                                nc.vector.tensor_mul(otmp, o_raw[0:64, :], rden)
                                nc.sync.dma_start(
                                    out=ostack[h // 2][64:128, qsl], in_=otmp)

                # ---------------- Phase C: projection ----------------
                with tc.tile_pool(name="wpst", bufs=2) as wpst, \
                     tc.tile_pool(name="wpr", bufs=1) as wprp, \
                     tc.tile_pool(name="ysb", bufs=3) as ysb, \
                     tc.tile_pool(name="y_ps", bufs=4, space="PSUM") as y_psp:
                    wp_r = []
                    for p in range(4):
                        st = wpst.tile([128, D], F32, tag="wpstage")
                        nc.sync.dma_start(out=st, in_=wp[p * 128:(p + 1) * 128, :])
                        wr = wprp.tile([128, D], F32R, tag=f"wp{p}")
                        nc.vector.tensor_copy(wr, st)
                        wp_r.append(wr)

                    for rt in range(NKT):
                        yt = ysb.tile([128, D], F32, tag="ytile")
                        for ncol in range(2):
                            ps = y_psp.tile([128, 512], F32, tag="yps")
                            for p in range(4):
                                nc.tensor.matmul(
                                    ps,
                                    ostack[p][:, rt * 128:(rt + 1) * 128],
                                    wp_r[p][:, ncol * 512:(ncol + 1) * 512],
                                    start=(p == 0), stop=(p == 3))
                            nc.vector.tensor_copy(
                                yt[:, ncol * 512:(ncol + 1) * 512], ps)
                        nc.sync.dma_start(
                            out=y[rt * 128:(rt + 1) * 128, :], in_=yt)
    nc.compile()
    return nc


_NC_CACHE = {}


def _get_nc(rep=1):
    if rep not in _NC_CACHE:
        _NC_CACHE[rep] = _build_nc(rep)
    return _NC_CACHE[rep]


def _run(in_maps):
    nc = _get_nc()
    return run_bass_kernel_spmd(nc, in_maps, core_ids=list(range(8)))


def _make_in_maps(x, w_qkv, w_proj):
    x = np.ascontiguousarray(x, dtype=np.float32)
    w_qkv = np.ascontiguousarray(w_qkv, dtype=np.float32)
    w_proj = np.ascontiguousarray(w_proj, dtype=np.float32)
    in_maps = []
    for c in range(8):
        b, g = divmod(c, 2)
        wq = w_qkv[:, g * 512:(g + 1) * 512]
        wk = w_qkv[:, D + g * 512:D + (g + 1) * 512]
        wvs = w_qkv[:, 2 * D + g * 512:2 * D + (g + 1) * 512]
        in_maps.append({
            "x": np.ascontiguousarray(x[b]),
            "wqk": np.ascontiguousarray(np.concatenate([wq, wk], axis=1)),
            "wv": np.ascontiguousarray(wvs),
            "wp": np.ascontiguousarray(w_proj[g * 512:(g + 1) * 512, :]),
        })
    return in_maps


def kernel(x, w_qkv, w_proj, b_proj):
    in_maps = _make_in_maps(x, w_qkv, w_proj)
    res = _run(in_maps)
    out = np.empty((B, N, D), dtype=np.float32)
    bp = np.asarray(b_proj, dtype=np.float32)
    for b in range(B):
        out[b] = res.results[2 * b]["y"] + res.results[2 * b + 1]["y"] + bp
    return out


if __name__ == "__main__":
    rng = np.random.default_rng(0)
    x = rng.standard_normal((B, N, D), dtype=np.float32)
    w_qkv = (rng.standard_normal((D, 3 * D), dtype=np.float32) * D ** -0.5)
    w_proj = (rng.standard_normal((D, D), dtype=np.float32) * D ** -0.5)
    b_proj = np.zeros(D, dtype=np.float32)
    out = kernel(x, w_qkv, w_proj, b_proj)
    print("ran; out shape", out.shape, "mean abs", np.abs(out).mean())



# revision 4
# speedup vs baseline: 1.9368x; 1.9368x over previous
"""Multi-head attention block (B=4, N=2048, D=1024, H=16) on 8 trn2 NeuronCores.

Sharding: core c -> (batch b = c//2, head-group g = c%2) with 8 heads per
group.  Each core computes q/k/v for its 8 heads over its batch, full
attention, and a partial projection y_part = attn_out_g @ w_proj[rows_g].
Host combines: out[b] = y_part[2b] + y_part[2b+1] + b_proj.

All device compute is bf16 (inputs converted host-side, including x
pre-transposed to x.T so the PE array never transposes activations).
q'/k' stay resident in SBUF as pair-stacked [128, seq] tiles; the odd
head's 64-partition half is rebased to partition 0 with an SBUF->SBUF DMA
(engines can't cross partitions; DMA can).  Softmax denominators ride as
a ones-column in the PV matmul (row 64 of o_ps) and bounce through DRAM
for the partition-broadcast.

Emission order keeps the PE busy end-to-end: qkv for pair 0 first, v
interleaved into the first attention unit, remaining qkv pairs drained
from a background queue between attention strips, projection at the end.
"""
import sys

sys.path.insert(0, "/opt/trn_rl_repo")

import numpy as np

import concourse.bass as bass
import concourse.mybir as mybir
import concourse.tile as tile
from concourse import bacc
from concourse.bass_utils import run_bass_kernel_spmd

F32 = mybir.dt.float32
BF16 = mybir.dt.bfloat16
AF = mybir.ActivationFunctionType

B = 4            # batch
N = 2048         # sequence length
D = 1024         # model dim
H = 16           # total heads
HD = 64          # head dim
HL = 8           # heads per core (local)
SCALE = HD ** -0.5

NKT = N // 128   # 16 key tiles
NDT = D // 128   # 8 d tiles
QW = 512         # q window (one attention unit)
NQW = N // QW    # 4


def _build_nc(rep=1):
    nc = bacc.Bacc(None, target_bir_lowering=False)

    xt = nc.declare_dram_parameter("xt", [D, N], BF16, isOutput=False)
    wqk = nc.declare_dram_parameter("wqk", [D, D], BF16, isOutput=False)
    wv = nc.declare_dram_parameter("wv", [D, 512], BF16, isOutput=False)
    wp = nc.declare_dram_parameter("wp", [512, D], BF16, isOutput=False)
    y = nc.declare_dram_parameter("y", [N, D], BF16, isOutput=True)

    with tile.TileContext(nc) as tc:
      with tc.tile_pool(name="dramp", bufs=1, space="DRAM") as dramp:
        # DRAM scratch for the denominator partition-broadcast bounce.
        dscr = [dramp.tile([NQW, QW], F32, tag=f"dscr{h}", name=f"dscr{h}")
                for h in range(HL)]
        for _rep in range(rep):
         with tc.tile_pool(name="ares", bufs=1) as ares, \
              tc.tile_pool(name="qkres", bufs=1) as qkres, \
              tc.tile_pool(name="vres", bufs=1) as vres, \
              tc.tile_pool(name="ores", bufs=1) as ores, \
              tc.tile_pool(name="pst", bufs=4) as pstp, \
              tc.tile_pool(name="epi", bufs=2) as epi, \
              tc.tile_pool(name="ytp", bufs=2) as ytp, \
              tc.tile_pool(name="s_ps", bufs=2, space="PSUM") as spsp, \
              tc.tile_pool(name="o_ps", bufs=1, space="PSUM") as opsp, \
              tc.tile_pool(name="bg_ps", bufs=2, space="PSUM") as bgpsp:

            # ---------------- input DMAs ----------------
            wqk_sb = [ares.tile([128, D], BF16, tag=f"wqk{dt}", name=f"wqk{dt}")
                      for dt in range(NDT)]
            xt_sb = [ares.tile([128, N], BF16, tag=f"xt{dt}", name=f"xt{dt}")
                     for dt in range(NDT)]
            wv_sb = [ares.tile([128, 512], BF16, tag=f"wv{dt}", name=f"wv{dt}")
                     for dt in range(NDT)]
            wp_sb = [ares.tile([128, D], BF16, tag=f"wp{p}", name=f"wp{p}")
                     for p in range(4)]

            # pair-0 weight columns + first x window land first so qk0 can
            # start ~5us in; the rest streams behind.  Input loads alternate
            # between the two HWDGE queues (SP + Activation) — ACT is idle
            # during the lead-in, so its queue is free bandwidth.
            _dmai = [0]

            def in_dma(out, in_):
                q = nc.sync if _dmai[0] % 2 == 0 else nc.scalar
                _dmai[0] += 1
                q.dma_start(out=out, in_=in_)

            for dt in range(NDT):
                in_dma(wqk_sb[dt][:, 0:128],
                       wqk[dt * 128:(dt + 1) * 128, 0:128])
                in_dma(wqk_sb[dt][:, 512:640],
                       wqk[dt * 128:(dt + 1) * 128, 512:640])
            for rw in range(4):
                for dt in range(NDT):
                    in_dma(xt_sb[dt][:, rw * 512:(rw + 1) * 512],
                           xt[dt * 128:(dt + 1) * 128, rw * 512:(rw + 1) * 512])
            for dt in range(NDT):
                in_dma(wqk_sb[dt][:, 128:512],
                       wqk[dt * 128:(dt + 1) * 128, 128:512])
                in_dma(wqk_sb[dt][:, 640:1024],
                       wqk[dt * 128:(dt + 1) * 128, 640:1024])
                in_dma(wv_sb[dt], wv[dt * 128:(dt + 1) * 128, :])
            for p in range(4):
                in_dma(wp_sb[p], wp[p * 128:(p + 1) * 128, :])

            # ---------------- resident tensors ----------------
            qres = [qkres.tile([128, N], BF16, tag=f"qr{hp}", name=f"qr{hp}")
                    for hp in range(4)]
            kres = [qkres.tile([128, N], BF16, tag=f"kr{hp}", name=f"kr{hp}")
                    for hp in range(4)]
            qodd = [qkres.tile([64, N], BF16, tag=f"qo{hp}", name=f"qo{hp}")
                    for hp in range(4)]
            kodd = [qkres.tile([64, N], BF16, tag=f"ko{hp}", name=f"ko{hp}")
                    for hp in range(4)]
            # v_aug[kt]: [128 keys, HL*(HD+1)]; per head 64 v cols + ones col
            v_aug = [vres.tile([128, HL * (HD + 1)], BF16, tag=f"va{kt}",
                               name=f"va{kt}") for kt in range(NKT)]
            ostack = [ores.tile([128, N], BF16, tag=f"os{p}", name=f"os{p}")
                      for p in range(4)]

            # ---------------- emit helpers ----------------
            def qk_window(sec, hp, rw):
                ps = bgpsp.tile([128, 512], F32, tag="bgps")
                col0 = sec * 512 + hp * 128
                for dt in range(NDT):
                    nc.tensor.matmul(
                        ps, wqk_sb[dt][:, col0:col0 + 128],
                        xt_sb[dt][:, rw * 512:(rw + 1) * 512],
                        start=(dt == 0), stop=(dt == NDT - 1))
                dst = (kres if sec else qres)[hp]
                nc.vector.tensor_copy(dst[:, rw * 512:(rw + 1) * 512], ps)

            def rebase(hp):
                nc.sync.dma_start(out=qodd[hp], in_=qres[hp][64:128, :])
                nc.sync.dma_start(out=kodd[hp], in_=kres[hp][64:128, :])

            def v_kt(kt):
                ps = bgpsp.tile([128, 512], F32, tag="bgps")
                for dt in range(NDT):
                    nc.tensor.matmul(
                        ps, xt_sb[dt][:, kt * 128:(kt + 1) * 128], wv_sb[dt],
                        start=(dt == 0), stop=(dt == NDT - 1))
                va3 = v_aug[kt].rearrange("p (h c) -> p h c", h=HL)
                nc.vector.tensor_copy(
                    va3[:, :, 0:HD], ps.rearrange("p (h c) -> p h c", h=HL))
                nc.vector.memset(va3[:, :, HD:HD + 1], 1.0)

            # background queue: (pair, cycles, fn).  qk for pair hp must be
            # emitted before the first (h=2hp) attention unit.
            bg = []
            for hp in range(1, 4):
                for rw in range(4):
                    bg.append((hp, 4300, lambda hp=hp, rw=rw: qk_window(1, hp, rw)))
                for rw in range(4):
                    bg.append((hp, 4300, lambda hp=hp, rw=rw: qk_window(0, hp, rw)))
                bg.append((hp, 100, lambda hp=hp: rebase(hp)))

            def drain_bg(budget):
                while bg and budget > 0:
                    _, cost, fn = bg.pop(0)
                    fn()
                    budget -= cost

            def drain_bg_pair(hp):
                while bg and bg[0][0] <= hp:
                    _, _, fn = bg.pop(0)
                    fn()

            # ---------------- lead: qk pair 0, first v tiles ----------------
            for rw in range(4):
                qk_window(1, 0, rw)
            for rw in range(4):
                qk_window(0, 0, rw)
            rebase(0)
            v_kt(0)
            v_kt(1)

            # ---------------- attention units ----------------
            for h in range(HL):
                hp, odd = divmod(h, 2)
                if not odd:
                    drain_bg_pair(hp)  # ensure this pair's q/k are emitted
                q_t = qodd[hp] if odd else qres[hp][0:64, :]
                k_t = kodd[hp] if odd else kres[hp][0:64, :]
                first_unit = (h == 0)
                for qw in range(NQW):
                    o_ps = opsp.tile([65, QW], F32, tag="ops")
                    q_ap = q_t[:, qw * QW:(qw + 1) * QW]
                    for s in range(8):
                        sp = spsp.tile([128, 1024], F32, tag="sps")
                        for i in range(2):
                            kt = 2 * s + i
                            nc.tensor.matmul(
                                sp[:, i * 512:(i + 1) * 512],
                                k_t[:, kt * 128:(kt + 1) * 128], q_ap,
                                start=True, stop=True)
                        p_sb = pstp.tile([128, 1024], BF16, tag="pst")
                        nc.scalar.activation(p_sb, sp, AF.Exp, scale=SCALE)
                        if first_unit and qw == 0:
                            for kt in (2 * s + 2, 2 * s + 3):
                                if kt < NKT:
                                    v_kt(kt)
                        for i in range(2):
                            kt = 2 * s + i
                            nc.tensor.matmul(
                                o_ps,
                                v_aug[kt][:, h * (HD + 1):(h + 1) * (HD + 1)],
                                p_sb[:, i * 512:(i + 1) * 512],
                                start=(kt == 0), stop=(kt == NKT - 1))
                        if not (first_unit and qw == 0):
                            drain_bg(700)
                    # epilogue: normalize rows by the ones-column sums
                    o_raw = epi.tile([65, QW], F32, tag="oraw")
                    nc.vector.tensor_copy(o_raw, o_ps)
                    nc.sync.dma_start(out=dscr[h][qw, :], in_=o_raw[64:65, :])
                    den_b = epi.tile([64, QW], F32, tag="denb")
                    bc_ap = bass.AP(
                        tensor=dscr[h].tensor,
                        offset=dscr[h].offset + qw * QW,
                        ap=[[0, 64], [1, QW]])
                    nc.sync.dma_start(out=den_b, in_=bc_ap)
                    rden = epi.tile([64, QW], F32, tag="rden")
                    nc.vector.reciprocal(rden, den_b)
                    qsl = slice(qw * QW, (qw + 1) * QW)
                    if not odd:
                        nc.vector.tensor_mul(
                            ostack[hp][0:64, qsl], o_raw[0:64, :], rden)
                    else:
                        otmp = epi.tile([64, QW], BF16, tag="otmp")
                        nc.vector.tensor_mul(otmp, o_raw[0:64, :], rden)
                        nc.sync.dma_start(
                            out=ostack[hp][64:128, qsl], in_=otmp)

            while bg:
                _, _, fn = bg.pop(0)
                fn()

            # ---------------- projection ----------------
            for rt in range(NKT):
                yt = ytp.tile([128, D], BF16, tag="yt")
                for ncol in range(2):
                    ps = bgpsp.tile([128, 512], F32, tag="bgps")
                    for p in range(4):
                        nc.tensor.matmul(
                            ps, ostack[p][:, rt * 128:(rt + 1) * 128],
                            wp_sb[p][:, ncol * 512:(ncol + 1) * 512],
                            start=(p == 0), stop=(p == 3))
                    nc.vector.tensor_copy(yt[:, ncol * 512:(ncol + 1) * 512], ps)
                nc.sync.dma_start(out=y[rt * 128:(rt + 1) * 128, :], in_=yt)
    nc.compile()
    return nc


_NC_CACHE = {}


def _get_nc(rep=1):
    if rep not in _NC_CACHE:
        _NC_CACHE[rep] = _build_nc(rep)
    return _NC_CACHE[rep]


def _run(in_maps):
    nc = _get_nc()
    return run_bass_kernel_spmd(nc, in_maps, core_ids=list(range(8)))


def _make_in_maps(x, w_qkv, w_proj):
    import ml_dtypes
    bf = ml_dtypes.bfloat16
    x = np.ascontiguousarray(x, dtype=np.float32)
    w_qkv = np.ascontiguousarray(w_qkv, dtype=np.float32)
    w_proj = np.ascontiguousarray(w_proj, dtype=np.float32)
    in_maps = []
    for c in range(8):
        b, g = divmod(c, 2)
        wq = w_qkv[:, g * 512:(g + 1) * 512]
        wk = w_qkv[:, D + g * 512:D + (g + 1) * 512]
        wvs = w_qkv[:, 2 * D + g * 512:2 * D + (g + 1) * 512]
        in_maps.append({
            "xt": np.ascontiguousarray(x[b].T).astype(bf),
            "wqk": np.ascontiguousarray(
                np.concatenate([wq, wk], axis=1)).astype(bf),
            "wv": np.ascontiguousarray(wvs).astype(bf),
            "wp": np.ascontiguousarray(w_proj[g * 512:(g + 1) * 512, :]).astype(bf),
        })
    return in_maps


def kernel(x, w_qkv, w_proj, b_proj):
    in_maps = _make_in_maps(x, w_qkv, w_proj)
    res = _run(in_maps)
    out = np.empty((B, N, D), dtype=np.float32)
    bp = np.asarray(b_proj, dtype=np.float32)
    for b in range(B):
        out[b] = (res.results[2 * b]["y"].astype(np.float32)
                  + res.results[2 * b + 1]["y"].astype(np.float32) + bp)
    return out


if __name__ == "__main__":
    rng = np.random.default_rng(0)
    x = rng.standard_normal((B, N, D), dtype=np.float32)
    w_qkv = (rng.standard_normal((D, 3 * D), dtype=np.float32) * D ** -0.5)
    w_proj = (rng.standard_normal((D, D), dtype=np.float32) * D ** -0.5)
    b_proj = np.zeros(D, dtype=np.float32)
    out = kernel(x, w_qkv, w_proj, b_proj)
    print("ran; out shape", out.shape, "mean abs", np.abs(out).mean())
